# revision 46
# baseline (speedup 1.0000x reference)
"""MLA-style attention-score kernel for Trainium2 (8 NeuronCores, SPMD).

Computes, for full inputs
    q_nope_with_k_up [B,H,S,r], compressed_kv [B,S,r], rope_attention [B,H,S,S],
    mask [B,S], have_causal_mask scalar:

    nope   = einsum("bhqr,bkr->bhqk", q, kv)
    scores = (rope + nope) / sqrt(192)   (+ causal / padding masks)
    attn   = softmax(scores, -1)
    out    = einsum("bhqk,bkr->bhqr", attn, kv)        [B,H,S,r] fp32

Sharding: B*H = 64 head-slots, 8 per core; core c owns batch c//2, heads
(c%2)*8..+8, with that batch's compressed_kv replicated on the core.

The causal mask (and the padding mask, when nonzero) is folded into a host-side
copy of rope, so the device kernel adds rope into the score PSUM with a single
DVE tensor_add per chunk and the PE never touches rope or mask data.
"""

import math
import os
import sys

import numpy as np

for _p in ("/opt/trn_rl_repo", "/root/.axon_site/_ro/trn_rl_repo"):
    if os.path.isdir(_p) and _p not in sys.path:
        sys.path.insert(0, _p)

import concourse.bass as bass
import concourse.mybir as mybir
import concourse.tile as tile
from concourse import bass_utils
from concourse.vector_clock import ScopedClock

B, H, S, R = 4, 16, 1024, 512
N_CORES = 8
HPC = (B * H) // N_CORES          # heads per core
P = 128                           # partition block
NSB = S // P                      # 8 s-blocks
NRB = R // P                      # 4 r-chunks
SCALE = 1.0 / math.sqrt(64 + 128)
NEG = -1e30
F32 = mybir.dt.float32
F32R = mybir.dt.float32r
BF16 = mybir.dt.bfloat16
AF = mybir.ActivationFunctionType

N_WARMUP = int(os.environ.get("ATTN_WARMUP", "8"))
ROPE_PREFETCH = int(os.environ.get("ATTN_ROPE_PF", "6"))
SKEW = int(os.environ.get("ATTN_SKEW", "3"))


class _TC(tile.TileContext):
    """TileContext whose end-of-kernel drain splits its semaphore waits
    across preceding NOPs (walrus in this image rejects >2 sync waits on
    one Drain)."""

    MAX_WAITS = 1

    def _drain_and_barrier(self, tick_clock, wait_clock):
        nop = self.nc.sync.nop(nofuse=True)
        wait_clock.add_sem_waits(
            nop.ins, ScopedClock({None: tick_clock.global_clock})
        )
        si = nop.ins.sync_info
        waits = list(si.on_wait) if si is not None else []
        if len(waits) > self.MAX_WAITS:
            nop.ins.sync_info = mybir.SyncInfo(
                on_wait=waits[: self.MAX_WAITS], on_update=[]
            )
            for i in range(self.MAX_WAITS, len(waits), self.MAX_WAITS):
                extra = self.nc.sync.nop(nofuse=True)
                extra.ins.sync_info = mybir.SyncInfo(
                    on_wait=waits[i : i + self.MAX_WAITS], on_update=[]
                )
        self.nc.sync.drain()
        self.nc.all_engine_barrier()
        popped = self.nc._tile_sem_poison_stack.pop()
        assert popped is self._sem_poison
        self.nc.clear_and_free_semaphores(list(self.sems.allocated().values()))
        self.nc.all_engine_barrier()


def _legalize_sync_waits(nc, max_waits=1):
    """walrus in this image allows only `max_waits` sync waits per
    instruction: move excess waits onto NOPs inserted just before the
    instruction on the same engine queue."""
    nid = 0
    for f in nc.m.functions:
        for blk in f.blocks:
            insts = blk.instructions
            out = []
            changed = False
            for inst in insts:
                si = inst.sync_info
                if si is not None and len(si.on_wait) > max_waits:
                    waits = list(si.on_wait)
                    n_pre = len(waits) - max_waits
                    for i in range(0, n_pre, max_waits):
                        nop = mybir.InstNoOp(
                            name=f"waitsplit_{nid}", ins=[], outs=[],
                            engine=inst.engine, bass_nofuse=True,
                            sync_info=mybir.SyncInfo(
                                on_wait=waits[i : min(i + max_waits, n_pre)],
                                on_update=[],
                            ),
                        )
                        nid += 1
                        out.append(nop)
                    inst.sync_info = mybir.SyncInfo(
                        on_wait=waits[n_pre:], on_update=list(si.on_update)
                    )
                    changed = True
                out.append(inst)
            if changed:
                blk.instructions = out


def build_program(causal: bool, hpc: int = HPC, legalize: bool = True):
    # q/rope/kv arrive pre-cast to bf16 (the on-device pipeline computed in
    # bf16 anyway, so casting on the host is precision-neutral and halves
    # the DRAM read traffic).
    nc = bass.Bass()
    q_d = nc.dram_tensor("q", [hpc, S, R], BF16, kind="ExternalInput")
    kv_d = nc.dram_tensor("kv", [S, R], BF16, kind="ExternalInput")
    rope_d = nc.dram_tensor("rope", [hpc, S, S], BF16, kind="ExternalInput")
    id_d = nc.dram_tensor("ident", [P, P], BF16, kind="ExternalInput")
    # out is the UNNORMALIZED attn@kv (bf16) plus per-row softmax sums;
    # the host divides. Relative precision is unchanged and the device
    # saves a reciprocal + scale per step.
    out_d = nc.dram_tensor("out", [hpc, S, R], BF16, kind="ExternalOutput")
    sums_d = nc.dram_tensor("sums", [hpc, S], F32, kind="ExternalOutput")

    def chunk_widths(kw):
        out = []
        rem = kw
        while rem > 512:
            take = 512 if rem - 512 >= 256 or rem - 512 == 0 else rem - 256
            out.append(take)
            rem -= take
        out.append(rem)
        return out

    with _TC(nc) as tc:
        with (
            tc.tile_pool(name="const", bufs=1) as const_pool,
            tc.tile_pool(name="kvn", bufs=1) as kvn_pool,
            tc.tile_pool(name="kvt", bufs=1) as kvt_pool,
            tc.tile_pool(name="qn", bufs=2) as qn_pool,
            tc.tile_pool(name="qt", bufs=2) as qt_pool,
            tc.tile_pool(name="rope", bufs=8) as rope_pool,
            tc.tile_pool(name="attn", bufs=5) as attn_pool,
            tc.tile_pool(name="sexp", bufs=4) as sexp_pool,
            tc.tile_pool(name="attnT", bufs=4) as attnT_pool,
            tc.tile_pool(name="sums", bufs=10) as sums_pool,
            tc.tile_pool(name="hsum", bufs=3) as hsum_pool,
            tc.tile_pool(name="outh", bufs=8) as out_pool,
            tc.tile_pool(name="psc", bufs=3, space="PSUM") as sc_pool,
            tc.tile_pool(name="ptr", bufs=3, space="PSUM") as tr_pool,
            tc.tile_pool(name="pav", bufs=2, space="PSUM") as av_pool,
        ):
            # ---- prologue. The PE warm-up multiplies a memset-zero tile so
            # it needs no DMA at all: it starts right at function entry and
            # keeps the PE busy (opening the HAM clock gate to 2.4GHz) until
            # kvA lands.
            ident_b = const_pool.tile([P, P], BF16, tag="identb", name="ident_b")
            nc.gpsimd.dma_start(ident_b[:], id_d[:, :])
            wz = const_pool.tile([P, P], F32, tag="wz", name="warmup_zero")
            nc.vector.memset(wz[:], 0.0)
            wu = sc_pool.tile([P, P], F32, tag="sc", name="warmup_ps")
            for _ in range(N_WARMUP):
                nc.tensor.matmul(
                    wu[:], wz[:].bitcast(F32R), wz[:].bitcast(F32R),
                    start=True, stop=True,
                )
            kv_src = kv_d.rearrange("(n p) r -> p n r", p=P)
            kvA = kvn_pool.tile([P, NSB // 2, R], BF16, tag="kvA", name="kvA")
            kvB = kvn_pool.tile([P, NSB // 2, R], BF16, tag="kvB", name="kvB")
            kv_half = lambda kb: (kvA if kb < NSB // 2 else kvB)[:, kb % (NSB // 2), :]

            ncopy = [0]

            def pcopy(dst, src):
                # PSUM -> SBUF evacuation, alternating DVE / ACT
                if ncopy[0] % 2 == 0:
                    nc.vector.tensor_copy(dst, src)
                else:
                    nc.scalar.copy(dst, src)
                ncopy[0] += 1

            def transpose_into(dst_tile, dst_off, blocks, idt):
                """PE-transpose [P,P] `blocks` (list of APs), writing the
                j-th transposed block at dst_tile[:, dst_off + j*P]."""
                psdt = blocks[0].dtype
                ps = tr_pool.tile([P, 512], psdt, tag="tr", name="tr_ps")
                for j, srcb in enumerate(blocks):
                    nc.tensor.matmul(
                        ps[:, j * P : (j + 1) * P], srcb, idt[:],
                        is_transpose=True,
                    )
                w = len(blocks) * P
                pcopy(dst_tile[:, dst_off : dst_off + w], ps[:, :w])

            kv_t = kvt_pool.tile([P, NRB, S], BF16)

            def transpose_block_into(dst_tile, m, src_blocks):
                """Transpose the 4 r-chunks of s-block m into dst_tile[:, rb,
                m*P:(m+1)*P] (one psum tile, one strided evacuation)."""
                ps = tr_pool.tile([P, 512], BF16, tag="tr", name="tr_ps")
                for rb in range(NRB):
                    nc.tensor.matmul(
                        ps[:, rb * P : (rb + 1) * P], src_blocks[rb], ident_b[:],
                        is_transpose=True,
                    )
                ps_v = ps[:, : NRB * P].rearrange("p (a b) -> p a b", a=NRB)
                pcopy(dst_tile[:, :, m * P : (m + 1) * P], ps_v)

            def emit_kvT_block(m):
                kvh = kvA if m < NSB // 2 else kvB
                transpose_block_into(
                    kv_t, m,
                    [kvh[:, m % (NSB // 2), rb * P : (rb + 1) * P]
                     for rb in range(NRB)],
                )

            def emit_kvT(g):
                for m in range(g * 4, g * 4 + 4):
                    emit_kvT_block(m)

            # ---- software-pipelined head/qb loop (stage 2 lags SKEW steps).
            # The last head runs its qb's in reverse so the pipeline drains
            # on the smallest (nk=1) step instead of the largest.
            steps = [(h, qb) for h in range(hpc) for qb in range(NSB)]
            if causal and hpc > 1:
                steps = steps[:-NSB] + steps[-NSB:][::-1]
            first_of_head = {}
            last_of_head = {}
            for _i, (_h, _qb) in enumerate(steps):
                first_of_head.setdefault(_h, _i)
                last_of_head[_h] = _i
            carry = {}   # step index -> dict of live tiles
            heads = {}   # h -> dict(qT=..., qA=..., qB=...)
            ropes = {}   # step index -> rope tile

            def q_issue(h, phase):
                # two 256KB SWDGE piece-DMAs per phase (0..3), interleaved
                # with rope loads so no single transfer backs up the queue
                q_src = q_d[h].rearrange("(n p) r -> p n r", p=P)
                hd = heads.setdefault(h, {})
                if phase == 0:
                    hd["qA"] = qn_pool.tile([P, NSB // 2, R], BF16,
                                            tag="qA", name="qA")
                if phase == 2:
                    hd["qB"] = qn_pool.tile([P, NSB // 2, R], BF16,
                                            tag="qB", name="qB")
                for j in (2 * phase, 2 * phase + 1):
                    dst = hd["qA"] if j < NSB // 2 else hd["qB"]
                    nc.gpsimd.dma_start(dst[:, j % (NSB // 2)], q_src[:, j])

            def build_qT(h, g):
                for m in range(g * 4, g * 4 + 4):
                    build_qT_block(h, m)

            def build_qT_block(h, m):
                if m == 0:
                    heads[h]["qT"] = qt_pool.tile([P, NRB, S], BF16, name="qT")
                qh = heads[h]["qA"] if m < NSB // 2 else heads[h]["qB"]
                transpose_block_into(
                    heads[h]["qT"], m,
                    [qh[:, m % (NSB // 2), rb * P : (rb + 1) * P]
                     for rb in range(NRB)],
                )

            def rope_load(i):
                h, qb = steps[i]
                nk = (qb + 1) if causal else NSB
                rope_t = rope_pool.tile([P, S], BF16, tag="rope", name="rope_t")
                nc.gpsimd.dma_start(rope_t[:, : nk * P],
                                    rope_d[h, qb * P : (qb + 1) * P, 0 : nk * P])
                ropes[i] = rope_t

            def stage1(i):
                h, qb = steps[i]
                if i == first_of_head[h]:
                    heads[h]["sums_out"] = hsum_pool.tile(
                        [P, NSB], F32, tag="hsum", name="hsum_t")
                if causal and h == 0:
                    # head 0 races the prologue DMA: transpose kv/q one
                    # s-block at a time, right before the step that needs it
                    emit_kvT_block(qb)
                    build_qT_block(0, qb)
                elif i == first_of_head[h]:
                    build_qT(h, 0)
                    if not causal or qb != 0:
                        build_qT(h, 1)
                qT = heads[h]["qT"]
                nk = (qb + 1) if causal else NSB
                kw = nk * P
                rope_t = ropes.pop(i)
                attn = attn_pool.tile([P, S], BF16, tag="attn", name="attn_t")
                sums_out = heads[h]["sums_out"]
                widths = chunk_widths(kw)
                sums = []
                c0 = 0
                for w in widths:
                    ps = sc_pool.tile([P, 512], F32, tag="sc", name="sc_ps")
                    for rb in range(NRB):
                        nc.tensor.matmul(
                            ps[:, :w],
                            qT[:, rb, qb * P : (qb + 1) * P],
                            kv_t[:, rb, c0 : c0 + w],
                            start=(rb == 0), stop=(rb == NRB - 1),
                        )
                    # rope (with masks pre-folded on host) rides DVE, not PE;
                    # writing to SBUF staging frees the PSUM bank immediately
                    se = sexp_pool.tile([P, 512], F32, tag="sexp", name="sexp_t")
                    nc.vector.tensor_add(
                        se[:, :w], ps[:, :w], rope_t[:, c0 : c0 + w]
                    )
                    if len(widths) == 1:
                        s_t = sums_out[:, qb : qb + 1]
                    else:
                        s_t = sums_pool.tile([P, 1], F32, tag="sums",
                                             name="sums_t")[:]
                    nc.scalar.activation(
                        attn[:, c0 : c0 + w], se[:, :w], AF.Exp,
                        scale=SCALE, accum_out=s_t,
                    )
                    sums.append(s_t)
                    c0 += w
                if len(sums) > 1:
                    nc.vector.tensor_add(
                        sums_out[:, qb : qb + 1], sums[0], sums[1]
                    )
                carry[i] = {"attn": attn, "nk": nk}
                if (causal and h != 0 and qb == 3
                        and steps[first_of_head[h]][1] == 0):
                    # emit after this step's scores so the PE isn't blocked
                    # on late qB arrivals before work that doesn't need them
                    build_qT(h, 1)

            def stage2(i):
                h, qb = steps[i]
                st = carry.pop(i)
                attn, nk = st["attn"], st["nk"]
                groups = []
                for g in range(0, nk, 4):
                    jcnt = min(4, nk - g)
                    at_g = attnT_pool.tile([P, 512], BF16,
                                           tag="attnT", name="attnT_t")
                    transpose_into(
                        at_g, 0,
                        [attn[:, (g + j) * P : (g + j + 1) * P] for j in range(jcnt)],
                        ident_b,
                    )
                    groups.append(at_g)
                av = av_pool.tile([P, R], F32, tag="av", name="av_ps")
                for kb in range(nk):
                    nc.tensor.matmul(
                        av[:],
                        groups[kb // 4][:, (kb % 4) * P : (kb % 4 + 1) * P],
                        kv_half(kb),
                        start=(kb == 0), stop=(kb == nk - 1),
                    )
                out_t = out_pool.tile([P, R], BF16, tag="outh", name="out_t")
                if i % 2 == 0:
                    nc.vector.tensor_copy(out_t[:], av[:])
                else:
                    nc.scalar.copy(out_t[:], av[:])
                # spread the last few stores over both rings so the final
                # drain isn't serialized on one dispatch queue
                ring = nc.scalar if (i >= len(steps) - 3 and i % 2 == 1) else nc.sync
                ring.dma_start(
                    out_d[h].rearrange("(n p) r -> p n r", p=P)[:, qb], out_t[:]
                )
                if i == last_of_head[h]:
                    nc.sync.dma_start(
                        sums_d[h].rearrange("(n p) -> p n", p=P),
                        heads[h]["sums_out"][:],
                    )
                    heads.pop(h)

            # prologue DMA order (one SWDGE queue, FIFO): interleave kv
            # pieces, q0 pieces and early ropes so the first steps' inputs
            # land as early as possible instead of behind one 2MB transfer.
            q0_src = q_d[0].rearrange("(n p) r -> p n r", p=P)
            q0A = qn_pool.tile([P, NSB // 2, R], BF16, tag="qA", name="qA")
            q0B = qn_pool.tile([P, NSB // 2, R], BF16, tag="qB", name="qB")
            heads[0] = {"qA": q0A, "qB": q0B}
            for m in range(NSB):
                kvh = kvA if m < NSB // 2 else kvB
                nc.gpsimd.dma_start(kvh[:, m % (NSB // 2)], kv_src[:, m])
                q0h = q0A if m < NSB // 2 else q0B
                nc.gpsimd.dma_start(q0h[:, m % (NSB // 2)], q0_src[:, m])
                if m < min(ROPE_PREFETCH, len(steps), 6):
                    rope_load(m)
            for j in range(6, min(ROPE_PREFETCH, len(steps))):
                rope_load(j)
            if not causal:
                emit_kvT(0)
                emit_kvT(1)
            for i in range(len(steps) + SKEW):
                if i < len(steps):
                    h, qb = steps[i]
                    if qb < 4 and h + 1 < hpc:
                        q_issue(h + 1, qb)
                    if i + ROPE_PREFETCH < len(steps):
                        rope_load(i + ROPE_PREFETCH)
                    stage1(i)
                if i >= SKEW:
                    stage2(i - SKEW)

    if legalize:
        _legalize_sync_waits(nc)
    return nc


_CACHE = {}


def _program(causal: bool):
    if causal not in _CACHE:
        _CACHE[causal] = build_program(causal)
    return _CACHE[causal]


def kernel(q_nope_with_k_up, compressed_kv, rope_attention, mask,
           have_causal_mask) -> np.ndarray:
    import ml_dtypes

    bf16 = np.dtype(ml_dtypes.bfloat16)
    q = np.asarray(q_nope_with_k_up, dtype=np.float32).astype(bf16)
    kv = np.asarray(compressed_kv, dtype=np.float32).astype(bf16)
    rope = np.asarray(rope_attention, dtype=np.float32)
    causal = bool(int(np.asarray(have_causal_mask)))

    fold_pad = mask is not None and np.asarray(mask).any()
    if fold_pad:
        m = np.asarray(mask)
        rope = rope + np.where(m, NEG, 0.0).astype(np.float32)[:, None, None, :]
    rope = rope.astype(bf16)
    if causal:
        neg = bf16.type(NEG)
        triu = np.triu(np.ones((P, P), dtype=np.bool_), 1)
        for qb in range(NSB):
            sl = slice(qb * P, (qb + 1) * P)
            rope[:, :, sl, sl][..., triu] = neg

    ident_np = np.eye(P, dtype=np.float32).astype(bf16)

    nc = _program(causal)
    in_maps = []
    for c in range(N_CORES):
        b, h0 = c // (H // HPC), (c % (H // HPC)) * HPC
        in_maps.append({
            "q": q[b, h0 : h0 + HPC],
            "kv": kv[b],
            "rope": rope[b, h0 : h0 + HPC],
            "ident": ident_np,
        })

    res = bass_utils.run_bass_kernel_spmd(nc, in_maps, core_ids=list(range(N_CORES)))

    out = np.empty((B, H, S, R), np.float32)
    for c in range(N_CORES):
        b, h0 = c // (H // HPC), (c % (H // HPC)) * HPC
        av = np.asarray(res.results[c]["out"]).astype(np.float32)
        sums = np.asarray(res.results[c]["sums"], dtype=np.float32)
        out[b, h0 : h0 + HPC] = av / sums[:, :, None]
    return out


# revision 52
# speedup vs baseline: 1.1034x; 1.1034x over previous
"""MLA-style attention-score kernel for Trainium2 (8 NeuronCores, SPMD).

Computes, for full inputs
    q_nope_with_k_up [B,H,S,r], compressed_kv [B,S,r], rope_attention [B,H,S,S],
    mask [B,S], have_causal_mask scalar:

    nope   = einsum("bhqr,bkr->bhqk", q, kv)
    scores = (rope + nope) / sqrt(192)   (+ causal / padding masks)
    attn   = softmax(scores, -1)
    out    = einsum("bhqk,bkr->bhqr", attn, kv)        [B,H,S,r] fp32

Sharding: B*H = 64 head-slots, 8 per core; core c owns batch c//2, heads
(c%2)*8..+8, with that batch's compressed_kv replicated on the core.

The causal mask (and the padding mask, when nonzero) is folded into a host-side
copy of rope, so the device kernel adds rope into the score PSUM with a single
DVE tensor_add per chunk and the PE never touches rope or mask data.
"""

import math
import os
import sys

import numpy as np

for _p in ("/opt/trn_rl_repo", "/root/.axon_site/_ro/trn_rl_repo"):
    if os.path.isdir(_p) and _p not in sys.path:
        sys.path.insert(0, _p)

import concourse.bass as bass
import concourse.mybir as mybir
import concourse.tile as tile
from concourse import bass_utils
from concourse.vector_clock import ScopedClock

B, H, S, R = 4, 16, 1024, 512
N_CORES = 8
HPC = (B * H) // N_CORES          # heads per core
P = 128                           # partition block
NSB = S // P                      # 8 s-blocks
NRB = R // P                      # 4 r-chunks
SCALE = 1.0 / math.sqrt(64 + 128)
NEG = -1e30
F32 = mybir.dt.float32
F32R = mybir.dt.float32r
BF16 = mybir.dt.bfloat16
AF = mybir.ActivationFunctionType

N_WARMUP = int(os.environ.get("ATTN_WARMUP", "8"))
ROPE_PREFETCH = int(os.environ.get("ATTN_ROPE_PF", "6"))
SKEW = int(os.environ.get("ATTN_SKEW", "3"))


class _TC(tile.TileContext):
    """TileContext whose end-of-kernel drain splits its semaphore waits
    across preceding NOPs (walrus in this image rejects >2 sync waits on
    one Drain)."""

    MAX_WAITS = 1

    def _drain_and_barrier(self, tick_clock, wait_clock):
        nop = self.nc.sync.nop(nofuse=True)
        wait_clock.add_sem_waits(
            nop.ins, ScopedClock({None: tick_clock.global_clock})
        )
        si = nop.ins.sync_info
        waits = list(si.on_wait) if si is not None else []
        if len(waits) > self.MAX_WAITS:
            nop.ins.sync_info = mybir.SyncInfo(
                on_wait=waits[: self.MAX_WAITS], on_update=[]
            )
            for i in range(self.MAX_WAITS, len(waits), self.MAX_WAITS):
                extra = self.nc.sync.nop(nofuse=True)
                extra.ins.sync_info = mybir.SyncInfo(
                    on_wait=waits[i : i + self.MAX_WAITS], on_update=[]
                )
        self.nc.sync.drain()
        self.nc.all_engine_barrier()
        popped = self.nc._tile_sem_poison_stack.pop()
        assert popped is self._sem_poison
        self.nc.clear_and_free_semaphores(list(self.sems.allocated().values()))
        self.nc.all_engine_barrier()


def _legalize_sync_waits(nc, max_waits=1):
    """walrus in this image allows only `max_waits` sync waits per
    instruction: move excess waits onto NOPs inserted just before the
    instruction on the same engine queue."""
    nid = 0
    for f in nc.m.functions:
        for blk in f.blocks:
            insts = blk.instructions
            out = []
            changed = False
            for inst in insts:
                si = inst.sync_info
                if si is not None and len(si.on_wait) > max_waits:
                    waits = list(si.on_wait)
                    n_pre = len(waits) - max_waits
                    for i in range(0, n_pre, max_waits):
                        nop = mybir.InstNoOp(
                            name=f"waitsplit_{nid}", ins=[], outs=[],
                            engine=inst.engine, bass_nofuse=True,
                            sync_info=mybir.SyncInfo(
                                on_wait=waits[i : min(i + max_waits, n_pre)],
                                on_update=[],
                            ),
                        )
                        nid += 1
                        out.append(nop)
                    inst.sync_info = mybir.SyncInfo(
                        on_wait=waits[n_pre:], on_update=list(si.on_update)
                    )
                    changed = True
                out.append(inst)
            if changed:
                blk.instructions = out


def build_program(causal: bool, hpc: int = HPC, legalize: bool = True):
    # q/rope/kv arrive pre-cast to bf16 (the on-device pipeline computed in
    # bf16 anyway, so casting on the host is precision-neutral and halves
    # the DRAM read traffic).
    nc = bass.Bass()
    q_d = nc.dram_tensor("q", [hpc, S, R], BF16, kind="ExternalInput")
    kv_d = nc.dram_tensor("kv", [S, R], BF16, kind="ExternalInput")
    rope_d = nc.dram_tensor("rope", [hpc, S, S], BF16, kind="ExternalInput")
    id_d = nc.dram_tensor("ident", [P, P], BF16, kind="ExternalInput")
    out_d = nc.dram_tensor("out", [hpc, S, R], BF16, kind="ExternalOutput")

    def chunk_widths(kw):
        out = []
        rem = kw
        while rem > 512:
            take = 512 if rem - 512 >= 256 or rem - 512 == 0 else rem - 256
            out.append(take)
            rem -= take
        out.append(rem)
        return out

    with _TC(nc) as tc:
        with (
            tc.tile_pool(name="const", bufs=1) as const_pool,
            tc.tile_pool(name="kvn", bufs=1) as kvn_pool,
            tc.tile_pool(name="kvt", bufs=1) as kvt_pool,
            tc.tile_pool(name="qn", bufs=2) as qn_pool,
            tc.tile_pool(name="qt", bufs=2) as qt_pool,
            tc.tile_pool(name="rope", bufs=8) as rope_pool,
            tc.tile_pool(name="attn", bufs=5) as attn_pool,
            tc.tile_pool(name="sexp", bufs=4) as sexp_pool,
            tc.tile_pool(name="attnT", bufs=4) as attnT_pool,
            tc.tile_pool(name="sums", bufs=10) as sums_pool,
            tc.tile_pool(name="outh", bufs=8) as out_pool,
            tc.tile_pool(name="psc", bufs=3, space="PSUM") as sc_pool,
            tc.tile_pool(name="ptr", bufs=3, space="PSUM") as tr_pool,
            tc.tile_pool(name="pav", bufs=2, space="PSUM") as av_pool,
        ):
            # ---- prologue. The PE warm-up multiplies a memset-zero tile so
            # it needs no DMA at all: it starts right at function entry and
            # keeps the PE busy (opening the HAM clock gate to 2.4GHz) until
            # kvA lands.
            ident_b = const_pool.tile([P, P], BF16, tag="identb", name="ident_b")
            nc.gpsimd.dma_start(ident_b[:], id_d[:, :])
            wz = const_pool.tile([P, P], F32, tag="wz", name="warmup_zero")
            nc.vector.memset(wz[:], 0.0)
            wu = sc_pool.tile([P, P], F32, tag="sc", name="warmup_ps")
            for _ in range(N_WARMUP):
                nc.tensor.matmul(
                    wu[:], wz[:].bitcast(F32R), wz[:].bitcast(F32R),
                    start=True, stop=True,
                )
            kv_src = kv_d.rearrange("(n p) r -> p n r", p=P)
            kvA = kvn_pool.tile([P, NSB // 2, R], BF16, tag="kvA", name="kvA")
            kvB = kvn_pool.tile([P, NSB // 2, R], BF16, tag="kvB", name="kvB")
            kv_half = lambda kb: (kvA if kb < NSB // 2 else kvB)[:, kb % (NSB // 2), :]

            ncopy = [0]

            def pcopy(dst, src):
                # PSUM -> SBUF evacuation, alternating DVE / ACT
                if ncopy[0] % 2 == 0:
                    nc.vector.tensor_copy(dst, src)
                else:
                    nc.scalar.copy(dst, src)
                ncopy[0] += 1

            def transpose_into(dst_tile, dst_off, blocks, idt):
                """PE-transpose [P,P] `blocks` (list of APs), writing the
                j-th transposed block at dst_tile[:, dst_off + j*P]."""
                psdt = blocks[0].dtype
                ps = tr_pool.tile([P, 512], psdt, tag="tr", name="tr_ps")
                for j, srcb in enumerate(blocks):
                    nc.tensor.matmul(
                        ps[:, j * P : (j + 1) * P], srcb, idt[:],
                        is_transpose=True,
                    )
                w = len(blocks) * P
                pcopy(dst_tile[:, dst_off : dst_off + w], ps[:, :w])

            kv_t = kvt_pool.tile([P, NRB, S], BF16)

            def transpose_block_into(dst_tile, m, src_blocks):
                """Transpose the 4 r-chunks of s-block m into dst_tile[:, rb,
                m*P:(m+1)*P] (one psum tile, one strided evacuation)."""
                ps = tr_pool.tile([P, 512], BF16, tag="tr", name="tr_ps")
                for rb in range(NRB):
                    nc.tensor.matmul(
                        ps[:, rb * P : (rb + 1) * P], src_blocks[rb], ident_b[:],
                        is_transpose=True,
                    )
                ps_v = ps[:, : NRB * P].rearrange("p (a b) -> p a b", a=NRB)
                pcopy(dst_tile[:, :, m * P : (m + 1) * P], ps_v)

            def emit_kvT_block(m):
                kvh = kvA if m < NSB // 2 else kvB
                transpose_block_into(
                    kv_t, m,
                    [kvh[:, m % (NSB // 2), rb * P : (rb + 1) * P]
                     for rb in range(NRB)],
                )

            def emit_kvT(g):
                for m in range(g * 4, g * 4 + 4):
                    emit_kvT_block(m)

            # ---- software-pipelined head/qb loop (stage 2 lags SKEW steps).
            # The last head runs its qb's in reverse so the pipeline drains
            # on the smallest (nk=1) step instead of the largest.
            steps = [(h, qb) for h in range(hpc) for qb in range(NSB)]
            if causal and hpc > 1:
                steps = steps[:-NSB] + steps[-NSB:][::-1]
            first_of_head = {}
            last_of_head = {}
            for _i, (_h, _qb) in enumerate(steps):
                first_of_head.setdefault(_h, _i)
                last_of_head[_h] = _i
            carry = {}   # step index -> dict of live tiles
            heads = {}   # h -> dict(qT=..., qA=..., qB=...)
            ropes = {}   # step index -> rope tile

            def q_issue(h, phase):
                # two 256KB SWDGE piece-DMAs per phase (0..3), interleaved
                # with rope loads so no single transfer backs up the queue
                q_src = q_d[h].rearrange("(n p) r -> p n r", p=P)
                hd = heads.setdefault(h, {})
                if phase == 0:
                    hd["qA"] = qn_pool.tile([P, NSB // 2, R], BF16,
                                            tag="qA", name="qA")
                if phase == 2:
                    hd["qB"] = qn_pool.tile([P, NSB // 2, R], BF16,
                                            tag="qB", name="qB")
                for j in (2 * phase, 2 * phase + 1):
                    dst = hd["qA"] if j < NSB // 2 else hd["qB"]
                    nc.gpsimd.dma_start(dst[:, j % (NSB // 2)], q_src[:, j])

            def build_qT(h, g):
                for m in range(g * 4, g * 4 + 4):
                    build_qT_block(h, m)

            def build_qT_block(h, m):
                if m == 0:
                    heads[h]["qT"] = qt_pool.tile([P, NRB, S], BF16, name="qT")
                qh = heads[h]["qA"] if m < NSB // 2 else heads[h]["qB"]
                transpose_block_into(
                    heads[h]["qT"], m,
                    [qh[:, m % (NSB // 2), rb * P : (rb + 1) * P]
                     for rb in range(NRB)],
                )

            def rope_load(i):
                h, qb = steps[i]
                nk = (qb + 1) if causal else NSB
                rope_t = rope_pool.tile([P, S], BF16, tag="rope", name="rope_t")
                nc.gpsimd.dma_start(rope_t[:, : nk * P],
                                    rope_d[h, qb * P : (qb + 1) * P, 0 : nk * P])
                ropes[i] = rope_t

            def stage1(i):
                h, qb = steps[i]
                if causal and h == 0:
                    # head 0 races the prologue DMA: transpose kv/q one
                    # s-block at a time, right before the step that needs it
                    emit_kvT_block(qb)
                    build_qT_block(0, qb)
                elif i == first_of_head[h]:
                    build_qT(h, 0)
                    if not causal or qb != 0:
                        build_qT(h, 1)
                qT = heads[h]["qT"]
                nk = (qb + 1) if causal else NSB
                kw = nk * P
                rope_t = ropes.pop(i)
                attn = attn_pool.tile([P, S], BF16, tag="attn", name="attn_t")
                sums = []
                c0 = 0
                for w in chunk_widths(kw):
                    ps = sc_pool.tile([P, 512], F32, tag="sc", name="sc_ps")
                    for rb in range(NRB):
                        nc.tensor.matmul(
                            ps[:, :w],
                            qT[:, rb, qb * P : (qb + 1) * P],
                            kv_t[:, rb, c0 : c0 + w],
                            start=(rb == 0), stop=(rb == NRB - 1),
                        )
                    # rope (with masks pre-folded on host) rides DVE, not PE;
                    # writing to SBUF staging frees the PSUM bank immediately
                    se = sexp_pool.tile([P, 512], F32, tag="sexp", name="sexp_t")
                    nc.vector.tensor_add(
                        se[:, :w], ps[:, :w], rope_t[:, c0 : c0 + w]
                    )
                    s_t = sums_pool.tile([P, 1], F32, tag="sums", name="sums_t")
                    nc.scalar.activation(
                        attn[:, c0 : c0 + w], se[:, :w], AF.Exp,
                        scale=SCALE, accum_out=s_t[:],
                    )
                    sums.append(s_t)
                    c0 += w
                if len(sums) > 1:
                    tot = sums_pool.tile([P, 1], F32, tag="sums", name="tot_t")
                    nc.vector.tensor_add(tot[:], sums[0][:], sums[1][:])
                    sums = [tot]
                carry[i] = {"attn": attn, "sum": sums[0], "nk": nk}
                if (causal and h != 0 and qb == 3
                        and steps[first_of_head[h]][1] == 0):
                    # emit after this step's scores so the PE isn't blocked
                    # on late qB arrivals before work that doesn't need them
                    build_qT(h, 1)

            def stage2(i):
                h, qb = steps[i]
                st = carry.pop(i)
                attn, nk = st["attn"], st["nk"]
                groups = []
                for g in range(0, nk, 4):
                    jcnt = min(4, nk - g)
                    at_g = attnT_pool.tile([P, 512], BF16,
                                           tag="attnT", name="attnT_t")
                    transpose_into(
                        at_g, 0,
                        [attn[:, (g + j) * P : (g + j + 1) * P] for j in range(jcnt)],
                        ident_b,
                    )
                    groups.append(at_g)
                av = av_pool.tile([P, R], F32, tag="av", name="av_ps")
                for kb in range(nk):
                    nc.tensor.matmul(
                        av[:],
                        groups[kb // 4][:, (kb % 4) * P : (kb % 4 + 1) * P],
                        kv_half(kb),
                        start=(kb == 0), stop=(kb == nk - 1),
                    )
                recip = sums_pool.tile([P, 1], F32, tag="recip", name="recip_t")
                nc.vector.reciprocal(recip[:], st["sum"][:])
                out_t = out_pool.tile([P, R], BF16, tag="outh", name="out_t")
                if i % 2 == 0:
                    nc.vector.tensor_scalar_mul(out_t[:], av[:], recip[:])
                else:
                    nc.scalar.activation(out_t[:], av[:], AF.Copy, scale=recip[:])
                # spread the last few stores over both rings so the final
                # drain isn't serialized on one dispatch queue
                ring = nc.scalar if (i >= len(steps) - 3 and i % 2 == 1) else nc.sync
                ring.dma_start(
                    out_d[h].rearrange("(n p) r -> p n r", p=P)[:, qb], out_t[:]
                )
                if i == last_of_head[h]:
                    heads.pop(h)

            # prologue DMA order (one SWDGE queue, FIFO): interleave kv
            # pieces, q0 pieces and early ropes so the first steps' inputs
            # land as early as possible instead of behind one 2MB transfer.
            q0_src = q_d[0].rearrange("(n p) r -> p n r", p=P)
            q0A = qn_pool.tile([P, NSB // 2, R], BF16, tag="qA", name="qA")
            q0B = qn_pool.tile([P, NSB // 2, R], BF16, tag="qB", name="qB")
            heads[0] = {"qA": q0A, "qB": q0B}
            for m in range(NSB):
                kvh = kvA if m < NSB // 2 else kvB
                nc.gpsimd.dma_start(kvh[:, m % (NSB // 2)], kv_src[:, m])
                q0h = q0A if m < NSB // 2 else q0B
                nc.gpsimd.dma_start(q0h[:, m % (NSB // 2)], q0_src[:, m])
                if m < min(ROPE_PREFETCH, len(steps), 6):
                    rope_load(m)
            for j in range(6, min(ROPE_PREFETCH, len(steps))):
                rope_load(j)
            if not causal:
                emit_kvT(0)
                emit_kvT(1)
            for i in range(len(steps) + SKEW):
                if i < len(steps):
                    h, qb = steps[i]
                    if qb < 4 and h + 1 < hpc:
                        q_issue(h + 1, qb)
                    if i + ROPE_PREFETCH < len(steps):
                        rope_load(i + ROPE_PREFETCH)
                    stage1(i)
                if i >= SKEW:
                    stage2(i - SKEW)

    if legalize:
        _legalize_sync_waits(nc)
    return nc


_CACHE = {}


def _program(causal: bool):
    if causal not in _CACHE:
        _CACHE[causal] = build_program(causal)
    return _CACHE[causal]


def kernel(q_nope_with_k_up, compressed_kv, rope_attention, mask,
           have_causal_mask) -> np.ndarray:
    import ml_dtypes

    bf16 = np.dtype(ml_dtypes.bfloat16)
    q = np.asarray(q_nope_with_k_up, dtype=np.float32).astype(bf16)
    kv = np.asarray(compressed_kv, dtype=np.float32).astype(bf16)
    rope = np.asarray(rope_attention, dtype=np.float32)
    causal = bool(int(np.asarray(have_causal_mask)))

    fold_pad = mask is not None and np.asarray(mask).any()
    if fold_pad:
        m = np.asarray(mask)
        rope = rope + np.where(m, NEG, 0.0).astype(np.float32)[:, None, None, :]
    rope = rope.astype(bf16)
    if causal:
        neg = bf16.type(NEG)
        triu = np.triu(np.ones((P, P), dtype=np.bool_), 1)
        for qb in range(NSB):
            sl = slice(qb * P, (qb + 1) * P)
            rope[:, :, sl, sl][..., triu] = neg

    ident_np = np.eye(P, dtype=np.float32).astype(bf16)

    nc = _program(causal)
    in_maps = []
    for c in range(N_CORES):
        b, h0 = c // (H // HPC), (c % (H // HPC)) * HPC
        in_maps.append({
            "q": q[b, h0 : h0 + HPC],
            "kv": kv[b],
            "rope": rope[b, h0 : h0 + HPC],
            "ident": ident_np,
        })

    res = bass_utils.run_bass_kernel_spmd(nc, in_maps, core_ids=list(range(N_CORES)))

    out = np.empty((B, H, S, R), np.float32)
    for c in range(N_CORES):
        b, h0 = c // (H // HPC), (c % (H // HPC)) * HPC
        out[b, h0 : h0 + HPC] = np.asarray(res.results[c]["out"]).astype(np.float32)
    return out
